# revision 1
# baseline (speedup 1.0000x reference)
"""Trainium2 Bass kernel for nn_EnhancedSpatioTemporalLayer.

out = relu( sum_k T_k @ relu(conv1x3(x)+b) @ theta_k ) + x @ res_w.T + res_b
with T_0 = I, T_1 = L, T_2 = 2L^2 - I  (cheb polys, L symmetric), refactored as

    t   = relu(conv(x) + conv_b)                       # [H, n, tau] per (b)
    va  = t @ (theta0 - theta2)                        # I-term
    vb  = t @ theta1
    vc  = t @ (2*theta2)
    gcn = relu(va + L @ vb + L^2 @ vc)
    out = gcn + res,  res = x @ res_w.T + res_b        # exact fp32 path

Sharding: data-parallel over batch, 2 batches per NeuronCore, 8 cores.
Per core the time axis is processed as two halves A=[0,144) B=[144,288)
that are paired into concurrent M=64 matmuls occupying disjoint PE array
quadrants (tile_position auto-derived from base partitions).
"""

import numpy as np
import ml_dtypes

B, FIN, N, T = 16, 3, 170, 288
H, O = 64, 64
NCORES = 8
BPC = B // NCORES       # batches per core
TH = T // 2             # tau half length (144)
NGRP = 2                # output-DMA groups per half
TG = TH // NGRP         # 72 tau per group per half
TC = 24                 # tau-pairs per chunk
NCHUNK = TG // TC       # chunks per group

bf16 = ml_dtypes.bfloat16

_cache = {}


def _build(reps=1, stage=4):
    import concourse.mybir as mybir
    import concourse.tile as tile
    from concourse import bacc

    f32 = mybir.dt.float32
    f32r = mybir.dt.float32r
    bft = mybir.dt.bfloat16
    Relu = mybir.ActivationFunctionType.Relu
    ADD = mybir.AluOpType.add

    nc = bacc.Bacc(None, target_bir_lowering=False)
    x_d = nc.declare_dram_parameter("x", [BPC, 9, T, N], f32r, isOutput=False)
    on_d = nc.declare_dram_parameter("ones", [2, TC, N], f32r, isOutput=False)
    wc_d = nc.declare_dram_parameter("wc", [128, 2 * H], f32r, isOutput=False)
    wa_d = nc.declare_dram_parameter("wa", [128, O], bft, isOutput=False)
    th_d = nc.declare_dram_parameter("th", [128, 2 * O], bft, isOutput=False)
    rw_d = nc.declare_dram_parameter("rw", [128, 2 * O], f32r, isOutput=False)
    lm0_d = nc.declare_dram_parameter("lm0", [128, N], bft, isOutput=False)
    l2m0_d = nc.declare_dram_parameter("l2m0", [128, N], bft, isOutput=False)
    lt_d = nc.declare_dram_parameter("lt", [128, N], bft, isOutput=False)
    lt2_d = nc.declare_dram_parameter("lt2", [128, N], bft, isOutput=False)
    cb_d = nc.declare_dram_parameter("cb", [128, 2], f32, isOutput=False)
    out_d = nc.declare_dram_parameter("out", [BPC, O, N, T], f32, isOutput=True)

    with tile.TileContext(nc) as tc:
        with (
            tc.tile_pool(name="const", bufs=1) as cp,
            tc.tile_pool(name="xp", bufs=2) as xpp,
            tc.tile_pool(name="tsb", bufs=2) as tsp,
            tc.tile_pool(name="v0sb", bufs=8) as v0sp,
            tc.tile_pool(name="xsb", bufs=4) as xsp,
            tc.tile_pool(name="osb", bufs=2) as osp,
            tc.tile_pool(name="rel", bufs=3) as relp,
            tc.tile_pool(name="cps", bufs=2, space="PSUM") as cpp,
            tc.tile_pool(name="v0a", bufs=1, space="PSUM") as v0ap,
            tc.tile_pool(name="v0b", bufs=1, space="PSUM") as v0bp,
            tc.tile_pool(name="xps", bufs=1, space="PSUM") as xpsp,
            tc.tile_pool(name="fin", bufs=2, space="PSUM") as finp,
            tc.tile_pool(name="res", bufs=1, space="PSUM") as resp,
        ):
            wc_t = cp.tile([128, 2 * H], f32r)
            wa_t = cp.tile([128, O], bft)
            th_t = cp.tile([128, 2 * O], bft)
            rw_t = cp.tile([128, 2 * O], f32r)
            lm0_t = cp.tile([128, N], bft)
            l2m0_t = cp.tile([128, N], bft)
            lt_t = cp.tile([128, N], bft)
            lt2_t = cp.tile([128, N], bft)
            cb_t = cp.tile([128, 2], f32)
            for t_, d_ in ((wc_t, wc_d), (wa_t, wa_d), (th_t, th_d),
                           (rw_t, rw_d), (lm0_t, lm0_d), (l2m0_t, l2m0_d),
                           (lt_t, lt_d), (lt2_t, lt2_d), (cb_t, cb_d)):
                nc.sync.dma_start(out=t_[:, :], in_=d_[:, :])

            def chunk_body(b, g, c, out_sb):
                tbA = g * TG + c * TC
                tbB = tbA + TH
                xp = xpp.tile([128, TC, N], f32r, name="xp")
                # rows 0:9 A-half patch (s1,s0,s2 x FIN, shift host-baked),
                # rows 9:18 B-half patch, rows 32:38 center taps for residual
                nc.sync.dma_start(out=xp[0:9, :, :], in_=x_d[b, :, tbA:tbA + TC, :])
                nc.sync.dma_start(out=xp[9:18, :, :], in_=x_d[b, :, tbB:tbB + TC, :])
                nc.sync.dma_start(out=xp[32:35, :, :], in_=x_d[b, 0:3, tbA:tbA + TC, :])
                nc.sync.dma_start(out=xp[35:38, :, :], in_=x_d[b, 0:3, tbB:tbB + TC, :])
                nc.sync.dma_start(out=xp[38:40, :, :], in_=on_d[:, :, :])

                # ---- temporal conv (float32r, both halves in one matmul via
                # block-diagonal weights) + fused bias+relu evac to bf16
                t_sb = tsp.tile([128, TC, N], bft, name="t_sb")
                for j in range(TC // 3 if stage >= 1 else 1):
                    cps_raw = cpp.tile([128, 512], f32, name="cps")
                    cps = cps_raw[:, 0:510].rearrange("p (t n) -> p t n", t=3)
                    rhs = xp[0:18, 3 * j:3 * j + 3, :]
                    nc.tensor.matmul(cps[:, :, :], wc_t[0:18, :], rhs,
                                     start=True, stop=True)
                    nc.scalar.activation(t_sb[:, 3 * j:3 * j + 3, :], cps[:, :, :],
                                         Relu, bias=cb_t[:, 0:1], scale=1.0)

                if stage < 1:
                    # minimal consumer so nothing is dead-code eliminated
                    nc.vector.tensor_scalar(
                        out_sb.rearrange("p n t -> p t n")[:, TC * c:TC * c + 3, :],
                        cps[:, :, :], cb_t[:, 1:2], None, ADD)
                    return
                if stage < 2:
                    nc.vector.tensor_scalar(
                        out_sb.rearrange("p n t -> p t n")[:, TC * c:TC * c + 3, :],
                        t_sb[:, 0:3, :], 0.0, None, ADD)
                    return
                # ---- theta matmuls (the layout flip): V = t^T @ [th1|2*th2]
                # A-half and B-half run in disjoint PE quadrants concurrently,
                # so they must drain to different PSUM banks (same-partition
                # same-bank concurrent drains are a fatal PSUM collision).
                vA = {}
                vB = {}
                xsb = {}
                v0psA = v0psB = xps = None
                for p in range(TC):
                    if p % 4 == 0:
                        v0psA = v0ap.tile([128, 4, 128], f32, name="v0psA")
                        v0psB = v0bp.tile([128, 4, 128], f32, name="v0psB")
                        xps = xpsp.tile([128, 4, 128], f32, name="xps")
                    nc.tensor.matmul(v0psA[:, p % 4, :], t_sb[0:64, p, 0:128],
                                     th_t[0:64, :], start=True, stop=True)
                    nc.tensor.matmul(v0psB[:, p % 4, :], t_sb[64:128, p, 0:128],
                                     th_t[64:128, :], start=True, stop=True)
                    nc.tensor.matmul(xps[0:42, p % 4, :], t_sb[0:64, p, 128:170],
                                     th_t[0:64, :], start=True, stop=True)
                    nc.tensor.matmul(xps[64:106, p % 4, :], t_sb[64:128, p, 128:170],
                                     th_t[64:128, :], start=True, stop=True)
                    if p % 4 == 3:
                        q = p // 4
                        vtA = v0sp.tile([128, 4, 128], bft, name="vtA")
                        vtB = v0sp.tile([128, 4, 128], bft, name="vtB")
                        nc.vector.tensor_copy(vtA[:, :, :], v0psA[:, :, :])
                        nc.vector.tensor_copy(vtB[:, :, :], v0psB[:, :, :])
                        vA[q] = vtA
                        vB[q] = vtB
                        xt = xsp.tile([128, 4, 128], bft, name="xt")
                        nc.scalar.copy(xt[0:42, :, :], xps[0:42, :, :])
                        nc.scalar.copy(xt[64:106, :, :], xps[64:106, :, :])
                        xsb[q] = xt

                if stage < 3 and stage != 5:
                    for q in range(3):
                        nc.vector.tensor_scalar(
                            out_sb.rearrange("p n t -> p t n")[:, TC * c + q, 0:128],
                            vA[q][:, 0, 0:128], 0.0, None, ADD)
                        nc.vector.tensor_scalar(
                            out_sb.rearrange("p n t -> p t n")[:, TC * c + q, 0:128],
                            vB[q][:, 1, 0:128], 0.0, None, ADD)
                        nc.vector.tensor_scalar(
                            out_sb.rearrange("p n t -> p t n")[:, TC * c + q, 0:128],
                            xsb[q][:, 2, 0:128], 0.0, None, ADD)
                    return
                # ---- cheb chain + relu + residual, per 3-pair psum bank
                for j in range(TC // 3):
                    fin_raw = finp.tile([128, 512], f32, name="fin")
                    fin = fin_raw[:, 0:510].rearrange("p (t n) -> p t n", t=3)
                    for half in (0, 1):
                        r0 = 64 * half
                        nc.tensor.matmul(fin[r0:r0 + 64, :, :], wa_t[r0:r0 + 64, :],
                                         t_sb[r0:r0 + 64, 3 * j:3 * j + 3, 0:N],
                                         start=True, stop=False, skip_group_check=True)
                    for half in (0, 1):
                        for tt in range(3):
                            p = 3 * j + tt
                            xt = xsb[p // 4]
                            r0 = 64 * half
                            rt = slice(r0, r0 + 42)
                            vt = (vA if half == 0 else vB)[p // 4]
                            vb = vt[:, p % 4, 0:64]
                            vc = vt[:, p % 4, 64:128]
                            vbm1 = xt[rt, p % 4, 0:64]
                            vcm1 = xt[rt, p % 4, 64:128]
                            o = fin[r0:r0 + 64, tt, :]
                            nc.tensor.matmul(o, vb, lm0_t[:, :], start=False,
                                             stop=False, skip_group_check=True)
                            nc.tensor.matmul(o, vc, l2m0_t[:, :], start=False,
                                             stop=False, skip_group_check=True)
                            if stage != 5:
                                nc.tensor.matmul(o, vbm1, lt_t[rt, :], start=False,
                                                 stop=False, skip_group_check=True)
                                nc.tensor.matmul(o, vcm1, lt2_t[rt, :], start=False,
                                                 stop=False, skip_group_check=True)
                    dst = out_sb.rearrange("p n t -> p t n")[:, TC * c + 3 * j:TC * c + 3 * j + 3, :]
                    if stage < 4 or stage == 5:
                        nc.vector.tensor_scalar(dst, fin[:, :, :], cb_t[:, 1:2], None, ADD)
                        continue
                    rel = relp.tile([128, 512], f32, name="rel")
                    nc.scalar.activation(rel[:, 0:510], fin_raw[:, 0:510], Relu)
                    res_raw = resp.tile([128, 512], f32, name="res")
                    res = res_raw[:, 0:510].rearrange("p (t n) -> p t n", t=3)
                    rhs = xp[32:40, 3 * j:3 * j + 3, :]
                    nc.tensor.matmul(res[:, :, :], rw_t[32:40, :], rhs,
                                     start=True, stop=True)
                    nc.vector.tensor_tensor(dst, rel[:, 0:510].rearrange("p (t n) -> p t n", t=3),
                                            res[:, :, :], ADD)

            def body():
                for b in range(BPC):
                    for g in range(NGRP):
                        out_sb = osp.tile([128, N, TG], f32, name="out_sb")
                        for c in range(NCHUNK):
                            chunk_body(b, g, c, out_sb)
                        if stage < 3:
                            continue
                        dst = out_d[b].rearrange("o n (h g t) -> h o g n t",
                                                 h=2, g=NGRP)[:, :, g, :, :]
                        nc.sync.dma_start(out=dst, in_=out_sb[:, :, :])

            if reps > 1:
                with tc.For_i(0, reps, 1):
                    body()
            else:
                body()

    nc.compile()
    return nc


def _prep(inputs):
    cheb = np.asarray(inputs["cheb"], np.float32)
    conv_w = np.asarray(inputs["conv_w"], np.float32)
    conv_b = np.asarray(inputs["conv_b"], np.float32)
    theta = np.asarray(inputs["theta"], np.float32)
    res_w = np.asarray(inputs["res_w"], np.float32)
    res_b = np.asarray(inputs["res_b"], np.float32)

    L = cheb[1]
    L2 = (cheb[2] + np.eye(N, dtype=np.float32)) / 2.0

    # block-diagonal conv weights: rows follow the xprep patch-row layout
    # (row = s-slot*FIN + f with s-slot order [s1, s0, s2]);
    # cols 0:64 feed the A-half output channels, 64:128 the B-half
    wc = np.zeros((128, 2 * H), np.float32)
    for slot, s in ((0, 1), (1, 0), (2, 2)):
        for f in range(FIN):
            wc[3 * slot + f, 0:H] = conv_w[:, f, 0, s]
            wc[9 + 3 * slot + f, H:2 * H] = conv_w[:, f, 0, s]

    wa = np.zeros((128, O), bf16)
    wa[0:64] = (theta[0] - theta[2]).astype(bf16)
    wa[64:128] = wa[0:64]

    th = np.zeros((128, 2 * O), bf16)
    th[0:64, 0:O] = theta[1].astype(bf16)
    th[0:64, O:2 * O] = (2.0 * theta[2]).astype(bf16)
    th[64:128] = th[0:64]

    rw = np.zeros((128, 2 * O), np.float32)
    rw[32:35, 0:O] = res_w.T
    rw[35:38, O:2 * O] = res_w.T
    rw[38, 0:O] = res_b
    rw[39, O:2 * O] = res_b

    lm0 = L[0:128].astype(bf16)
    l2m0 = L2[0:128].astype(bf16)
    lt = np.zeros((128, N), bf16)
    lt[0:42] = L[128:N].astype(bf16)
    lt[64:106] = L[128:N].astype(bf16)
    lt2 = np.zeros((128, N), bf16)
    lt2[0:42] = L2[128:N].astype(bf16)
    lt2[64:106] = L2[128:N].astype(bf16)

    cb = np.zeros((128, 2), np.float32)
    cb[0:64, 0] = conv_b
    cb[64:128, 0] = conv_b
    cb[0:64, 1] = res_b
    cb[64:128, 1] = res_b

    ones = np.ones((2, TC, N), np.float32)
    return {"wc": wc, "wa": wa, "th": th, "rw": rw, "lm0": lm0,
            "l2m0": l2m0, "lt": lt, "lt2": lt2, "cb": cb, "ones": ones}


def _prep_x(x):
    """[B, FIN, N, T] -> [B, 9, T, N] patch rows with temporal shift baked in.

    row = slot*FIN + f, slot order [s1(center), s0(tau-1), s2(tau+1)],
    zero-padded at the tau edges."""
    Bn = x.shape[0]
    xt = np.transpose(x, (0, 1, 3, 2))          # [B, FIN, T, N]
    xprep = np.zeros((Bn, 9, T, N), np.float32)
    xprep[:, 0:3] = xt                          # s=1 (center)
    xprep[:, 3:6, 1:T] = xt[:, :, 0:T - 1]      # s=0 reads tau-1
    xprep[:, 6:9, 0:T - 1] = xt[:, :, 1:T]      # s=2 reads tau+1
    return xprep


def kernel(**inputs):
    from concourse.bass_utils import run_bass_kernel_spmd

    if "nc" not in _cache:
        _cache["nc"] = _build(1)
    nc = _cache["nc"]

    x = np.asarray(inputs["x"], np.float32)
    weights = _prep(inputs)
    xprep = _prep_x(x)
    in_maps = []
    for cid in range(NCORES):
        m = dict(weights)
        m["x"] = np.ascontiguousarray(xprep[cid * BPC:(cid + 1) * BPC])
        in_maps.append(m)
    res = run_bass_kernel_spmd(nc, in_maps, list(range(NCORES)), trace=False)
    out = np.concatenate([res.results[cid]["out"] for cid in range(NCORES)], axis=0)
    return out.astype(np.float32)



# revision 3
# speedup vs baseline: 1.9636x; 1.9636x over previous
"""Trainium2 Bass kernel for nn_EnhancedSpatioTemporalLayer, v3.

out = relu( sum_k T_k @ relu(conv1x3(x)+b) @ theta_k ) + x @ res_w.T + res_b
with T_0 = I, T_1 = L, T_2 = 2L^2 - I (cheb polys, L symmetric), refactored as

    t    = relu(conv(x) + conv_b)                      # [H, n, tau]
    V    = t^T @ [theta0-theta2 | theta1 | 2*theta2]   # [n, 192] per tau
    finT = I @ V_a + L @ V_b + L^2 @ V_c               # [n, tau, O] transposed
    out  = relu(finT) + resT,  resT = x^T @ res_w.T + res_b

The chain runs TRANSPOSED (vertices on partitions) so the L/L2/I weights are
CONSTANT stationaries streaming (tau-slot, o) blocks: 10 matmuls per 4-tau
block instead of 16 per-tau stationary swaps; the 128-col main chain weights
get fast weight load. Residual reuses the conv patch rows (s1/s2 slots are
the centers of pairs q0/q0+1), so one const-rhs matmul covers a block.

Sharding: data-parallel over batch, 2 batches per core, 8 cores.
Output layout: per (b, g): main [128 n, NBLK*4*O] + tail [42 n, ...], slot
interleave (pair-in-block, half); host reassembles.
"""

import numpy as np
import ml_dtypes

B, FIN, N, T = 16, 3, 170, 288
H, O = 64, 64
NCORES = 8
BPC = B // NCORES       # batches per core
TH = T // 2             # tau half length (144)
NGRP = 4                # groups per half (1 chunk per group)
TG = TH // NGRP         # 36 tau-pairs per group
TC = TG
NBLK = TC // 2          # 2-pair (4-tau) blocks per chunk (18)
NT = N - 128            # vertex tail size (42)
FG = NBLK * 4 * O       # out free size per group per partition (4608)

bf16 = ml_dtypes.bfloat16

_cache = {}


def _build(reps=1, stage=4, timing=False):
    import concourse.mybir as mybir
    import concourse.tile as tile
    from concourse import bacc

    f32 = mybir.dt.float32
    f32r = mybir.dt.float32r
    bft = mybir.dt.bfloat16
    Relu = mybir.ActivationFunctionType.Relu
    ADD = mybir.AluOpType.add
    MAX = mybir.AluOpType.max

    nc = bacc.Bacc(None, target_bir_lowering=False)
    if timing:
        x_d = nc.dram_tensor("x", [BPC, NGRP, 20, TC, N], f32r,
                             kind="Internal").ap()
    else:
        x_d = nc.declare_dram_parameter(
            "x", [BPC, NGRP, 20, TC, N], f32r, isOutput=False)
    wc_d = nc.declare_dram_parameter("wc", [128, 2 * H], f32r, isOutput=False)
    th_d = nc.declare_dram_parameter("th", [128, 192], bft, isOutput=False)
    rb_d = nc.declare_dram_parameter("rb", [128, 256], f32r, isOutput=False)
    ln0_d = nc.declare_dram_parameter("ln0", [128, 128], bft, isOutput=False)
    l2n0_d = nc.declare_dram_parameter("l2n0", [128, 128], bft, isOutput=False)
    ltn0_d = nc.declare_dram_parameter("ltn0", [NT, 128], bft, isOutput=False)
    l2tn0_d = nc.declare_dram_parameter("l2tn0", [NT, 128], bft, isOutput=False)
    ln1_d = nc.declare_dram_parameter("ln1", [128, NT], bft, isOutput=False)
    l2n1_d = nc.declare_dram_parameter("l2n1", [128, NT], bft, isOutput=False)
    ltn1_d = nc.declare_dram_parameter("ltn1", [NT, NT], bft, isOutput=False)
    l2tn1_d = nc.declare_dram_parameter("l2tn1", [NT, NT], bft, isOutput=False)
    eyn0_d = nc.declare_dram_parameter("eyn0", [128, 128], bft, isOutput=False)
    eyn1_d = nc.declare_dram_parameter("eyn1", [NT, NT], bft, isOutput=False)
    cb_d = nc.declare_dram_parameter("cb", [128, 2], f32, isOutput=False)
    if timing:
        out_d = nc.dram_tensor("out", [BPC, NGRP, 128, FG], f32,
                               kind="Internal").ap()
        out2_d = nc.dram_tensor("out2", [BPC, NGRP, NT, FG], f32,
                                kind="Internal").ap()
        tick_d = nc.declare_dram_parameter("tick", [128, 2], f32, isOutput=True)
    else:
        out_d = nc.declare_dram_parameter("out", [BPC, NGRP, 128, FG], f32,
                                          isOutput=True)
        out2_d = nc.declare_dram_parameter("out2", [BPC, NGRP, NT, FG], f32,
                                           isOutput=True)

    with tile.TileContext(nc) as tc:
        with (
            tc.tile_pool(name="const", bufs=1) as cp,
            tc.tile_pool(name="xp", bufs=2) as xpp,
            tc.tile_pool(name="tsb", bufs=2) as tsp,
            tc.tile_pool(name="vsb", bufs=3) as vsp,
            tc.tile_pool(name="osbm", bufs=2) as osmp,
            tc.tile_pool(name="osbt", bufs=2) as ostp,
            tc.tile_pool(name="rel", bufs=3) as relp,
            tc.tile_pool(name="cps", bufs=2, space="PSUM") as cpp,
            tc.tile_pool(name="vam", bufs=1, space="PSUM") as vamp,
            tc.tile_pool(name="vbm", bufs=1, space="PSUM") as vbmp,
            tc.tile_pool(name="vat", bufs=1, space="PSUM") as vatp,
            tc.tile_pool(name="vbt", bufs=1, space="PSUM") as vbtp,
            tc.tile_pool(name="fin", bufs=1, space="PSUM") as finp,
            tc.tile_pool(name="res", bufs=1, space="PSUM") as resp,
        ):
            wc_t = cp.tile([128, 2 * H], f32r)
            th_t = cp.tile([128, 192], bft)
            rb_t = cp.tile([128, 256], f32r)
            ln0_t = cp.tile([128, 128], bft)
            l2n0_t = cp.tile([128, 128], bft)
            ltn0_t = cp.tile([NT, 128], bft)
            l2tn0_t = cp.tile([NT, 128], bft)
            ln1_t = cp.tile([128, NT], bft)
            l2n1_t = cp.tile([128, NT], bft)
            ltn1_t = cp.tile([NT, NT], bft)
            l2tn1_t = cp.tile([NT, NT], bft)
            eyn0_t = cp.tile([128, 128], bft)
            eyn1_t = cp.tile([NT, NT], bft)
            cb_t = cp.tile([128, 2], f32)
            for t_, d_ in ((wc_t, wc_d), (th_t, th_d), (rb_t, rb_d),
                           (ln0_t, ln0_d), (l2n0_t, l2n0_d), (ltn0_t, ltn0_d),
                           (l2tn0_t, l2tn0_d), (ln1_t, ln1_d), (l2n1_t, l2n1_d),
                           (ltn1_t, ltn1_d), (l2tn1_t, l2tn1_d),
                           (eyn0_t, eyn0_d), (eyn1_t, eyn1_d), (cb_t, cb_d)):
                nc.sync.dma_start(out=t_[:, :], in_=d_[:, :])

            def block_body(k, xp, t_sb, osb_m, osb_t):
                q0 = 2 * k
                # ---- theta: V = t^T @ [tha|th1|2th2]; A/B halves drain to
                # separate PSUM banks (concurrent quadrant drains must never
                # share a bank at the same partitions); slot = 2*ph + h
                vm = {0: vamp.tile([128, 2, 192], f32, name="vam"),
                      1: vbmp.tile([128, 2, 192], f32, name="vbm")}
                vt = {0: vatp.tile([128, 2, 192], f32, name="vat"),
                      1: vbtp.tile([128, 2, 192], f32, name="vbt")}
                for ph in range(2):
                    q = q0 + ph
                    for h in range(2):
                        r0 = 64 * h
                        nc.tensor.matmul(vm[h][:, ph, :],
                                         t_sb[r0:r0 + 64, q, 0:128],
                                         th_t[r0:r0 + 64, :],
                                         start=True, stop=True)
                        nc.tensor.matmul(vt[h][0:NT, ph, :],
                                         t_sb[r0:r0 + 64, q, 128:N],
                                         th_t[r0:r0 + 64, :],
                                         start=True, stop=True)
                # ---- evac to one bf16 tile: main cols 0:192 = [va|vb|vc],
                # tail cols 192:384
                vsb = vsp.tile([128, 4, 384], bft, name="vsb")
                nc.vector.tensor_copy(vsb[:, 0:4:2, 0:192], vm[0][:, :, :])
                nc.scalar.copy(vsb[:, 1:4:2, 0:192], vm[1][:, :, :])
                nc.scalar.copy(vsb[0:NT, 0:4:2, 192:384], vt[0][0:NT, :, :])
                nc.vector.tensor_copy(vsb[0:NT, 1:4:2, 192:384], vt[1][0:NT, :, :])

                if stage < 3:
                    nc.vector.tensor_scalar(
                        osb_m[:, k, :, :], vsb[:, :, 0:64],
                        cb_t[:, 1:2], None, ADD)
                    nc.vector.tensor_scalar(
                        osb_t[0:NT, k, :, :], vsb[0:NT, :, 192:256],
                        cb_t[:, 1:2], None, ADD)
                    return
                # ---- chain: finT[n, slot, o] = I@Va + L@Vb + L2@Vc with
                # constant stationaries over all 4 slots per matmul
                fin_raw = finp.tile([128, 512], f32, name="fin")
                fm = fin_raw[:, 0:256].rearrange("p (s o) -> p s o", s=4)
                ft = fin_raw[:, 256:512].rearrange("p (s o) -> p s o", s=4)
                nc.tensor.matmul(fm[:, :, :], ln0_t[:, :], vsb[:, :, 64:128],
                                 start=True, stop=False, skip_group_check=True)
                nc.tensor.matmul(fm[:, :, :], l2n0_t[:, :], vsb[:, :, 128:192],
                                 start=False, stop=False, skip_group_check=True)
                nc.tensor.matmul(fm[:, :, :], eyn0_t[:, :], vsb[:, :, 0:64],
                                 start=False, stop=False, skip_group_check=True)
                nc.tensor.matmul(fm[:, :, :], ltn0_t[0:NT, :],
                                 vsb[0:NT, :, 256:320],
                                 start=False, stop=False, skip_group_check=True)
                nc.tensor.matmul(fm[:, :, :], l2tn0_t[0:NT, :],
                                 vsb[0:NT, :, 320:384],
                                 start=False, stop=True, skip_group_check=True)
                nc.tensor.matmul(ft[0:NT, :, :], ln1_t[:, :],
                                 vsb[:, :, 64:128],
                                 start=True, stop=False, skip_group_check=True)
                nc.tensor.matmul(ft[0:NT, :, :], l2n1_t[:, :],
                                 vsb[:, :, 128:192],
                                 start=False, stop=False, skip_group_check=True)
                nc.tensor.matmul(ft[0:NT, :, :], ltn1_t[0:NT, :],
                                 vsb[0:NT, :, 256:320],
                                 start=False, stop=False, skip_group_check=True)
                nc.tensor.matmul(ft[0:NT, :, :], l2tn1_t[0:NT, :],
                                 vsb[0:NT, :, 320:384],
                                 start=False, stop=False, skip_group_check=True)
                nc.tensor.matmul(ft[0:NT, :, :], eyn1_t[0:NT, :],
                                 vsb[0:NT, :, 192:256],
                                 start=False, stop=True, skip_group_check=True)
                # ---- residual (transposed): const rhs; xp s1/s2 slots at
                # pair q0 are the centers of pairs q0 / q0+1
                res_raw = resp.tile([128, 512], f32, name="res")
                nc.tensor.matmul(res_raw[:, 0:256], xp[0:20, q0, 0:128],
                                 rb_t[0:20, :], start=True, stop=True)
                nc.tensor.matmul(res_raw[0:NT, 256:512], xp[0:20, q0, 128:N],
                                 rb_t[0:20, :], start=True, stop=True)
                if stage < 4:
                    nc.vector.tensor_scalar(osb_m[:, k, :, :], fm[:, :, :],
                                            cb_t[:, 1:2], None, ADD)
                    nc.vector.tensor_scalar(osb_t[0:NT, k, :, :],
                                            ft[0:NT, :, :],
                                            cb_t[:, 1:2], None, ADD)
                    return
                # ---- final: relu(fin) + res
                rel = relp.tile([128, 512], f32, name="rel")
                nc.scalar.activation(rel[:, 0:256], fin_raw[:, 0:256], Relu)
                nc.scalar.activation(rel[0:NT, 256:512],
                                     fin_raw[0:NT, 256:512], Relu)
                nc.vector.tensor_tensor(
                    osb_m[:, k, :, :],
                    rel[:, 0:256].rearrange("p (s o) -> p s o", s=4),
                    res_raw[:, 0:256].rearrange("p (s o) -> p s o", s=4), ADD)
                nc.vector.tensor_tensor(
                    osb_t[0:NT, k, :, :],
                    rel[0:NT, 256:512].rearrange("p (s o) -> p s o", s=4),
                    res_raw[0:NT, 256:512].rearrange("p (s o) -> p s o", s=4),
                    ADD)

            def body():
                for b in range(BPC):
                    for g in range(NGRP):
                        xp = xpp.tile([128, TC, N], f32r, name="xp")
                        nc.sync.dma_start(out=xp[0:20, :, :],
                                          in_=x_d[b, g, :, :, :])
                        t_sb = tsp.tile([128, TC, N], bft, name="t_sb")
                        for j in range(TC // 3):
                            cps_raw = cpp.tile([128, 512], f32, name="cps")
                            cps = cps_raw[:, 0:510].rearrange(
                                "p (t n) -> p t n", t=3)
                            nc.tensor.matmul(cps[:, :, :], wc_t[0:18, :],
                                             xp[0:18, 3 * j:3 * j + 3, :],
                                             start=True, stop=True)
                            nc.scalar.activation(
                                t_sb[:, 3 * j:3 * j + 3, :], cps[:, :, :],
                                Relu, bias=cb_t[:, 0:1], scale=1.0)
                        osb_m = osmp.tile([128, NBLK, 4, O], f32, name="osb_m")
                        osb_t = ostp.tile([128, NBLK, 4, O], f32, name="osb_t")
                        for k in range(NBLK):
                            block_body(k, xp, t_sb, osb_m, osb_t)
                        nc.gpsimd.dma_start(
                            out=out_d[b, g, :, :],
                            in_=osb_m.rearrange("p k s o -> p (k s o)"))
                        nc.gpsimd.dma_start(
                            out=out2_d[b, g, :, :],
                            in_=osb_t[0:NT].rearrange("p k s o -> p (k s o)"))

            if reps > 1:
                with tc.For_i(0, reps, 1):
                    body()
            else:
                body()
            if timing:
                nc.sync.dma_start(out=tick_d[:, :], in_=cb_t[:, :])

    nc.compile()
    return nc


def _prep(inputs):
    cheb = np.asarray(inputs["cheb"], np.float32)
    conv_w = np.asarray(inputs["conv_w"], np.float32)
    conv_b = np.asarray(inputs["conv_b"], np.float32)
    theta = np.asarray(inputs["theta"], np.float32)
    res_w = np.asarray(inputs["res_w"], np.float32)
    res_b = np.asarray(inputs["res_b"], np.float32)

    L = cheb[1]
    L2 = (cheb[2] + np.eye(N, dtype=np.float32)) / 2.0

    # block-diagonal conv weights (rows: [s1,s0,s2] x FIN, A cols 0:64,
    # B cols 64:128)
    wc = np.zeros((128, 2 * H), np.float32)
    for slot, s in ((0, 1), (1, 0), (2, 2)):
        for f in range(FIN):
            wc[3 * slot + f, 0:H] = conv_w[:, f, 0, s]
            wc[9 + 3 * slot + f, H:2 * H] = conv_w[:, f, 0, s]

    th = np.zeros((128, 192), bf16)
    th[0:64, 0:64] = (theta[0] - theta[2]).astype(bf16)
    th[0:64, 64:128] = theta[1].astype(bf16)
    th[0:64, 128:192] = (2.0 * theta[2]).astype(bf16)
    th[64:128] = th[0:64]

    # residual rhs: rows follow the xp patch rows; slot s gets cols 64s:64s+64
    rb = np.zeros((128, 256), np.float32)
    rb[0:3, 0:64] = res_w.T            # s1-A -> slot 0 (pair q0, half A)
    rb[9:12, 64:128] = res_w.T         # s1-B -> slot 1
    rb[6:9, 128:192] = res_w.T         # s2-A -> slot 2 (pair q0+1, half A)
    rb[15:18, 192:256] = res_w.T       # s2-B -> slot 3
    rb[18, 0:64] = res_b
    rb[18, 128:192] = res_b
    rb[19, 64:128] = res_b
    rb[19, 192:256] = res_b

    w = {
        "wc": wc, "th": th, "rb": rb,
        "ln0": L[0:128, 0:128].astype(bf16),
        "l2n0": L2[0:128, 0:128].astype(bf16),
        "ltn0": L[128:N, 0:128].astype(bf16),
        "l2tn0": L2[128:N, 0:128].astype(bf16),
        "ln1": L[0:128, 128:N].astype(bf16),
        "l2n1": L2[0:128, 128:N].astype(bf16),
        "ltn1": L[128:N, 128:N].astype(bf16),
        "l2tn1": L2[128:N, 128:N].astype(bf16),
        "eyn0": np.eye(128, dtype=bf16),
        "eyn1": np.eye(NT, dtype=bf16),
    }
    cb = np.zeros((128, 2), np.float32)
    cb[0:64, 0] = conv_b
    cb[64:128, 0] = conv_b
    w["cb"] = cb
    return w


def _prep_x(x):
    """[B, FIN, N, T] -> [B, NGRP, 20, TC, N] per-group patch blocks.

    rows 0:9 A-half patch at pairs [g*TC, g*TC+TC), rows 9:18 B-half,
    rows 18:20 ones. Patch row = slot*FIN + f, slot order [s1, s0, s2],
    zero-padded at global tau edges."""
    Bn = x.shape[0]
    xt = np.transpose(x, (0, 1, 3, 2))          # [B, FIN, T, N]
    xprep = np.zeros((Bn, 9, T, N), np.float32)
    xprep[:, 0:3] = xt
    xprep[:, 3:6, 1:T] = xt[:, :, 0:T - 1]
    xprep[:, 6:9, 0:T - 1] = xt[:, :, 1:T]
    xall = np.ones((Bn, NGRP, 20, TC, N), np.float32)
    for g in range(NGRP):
        tbA = g * TC
        tbB = tbA + TH
        xall[:, g, 0:9] = xprep[:, :, tbA:tbA + TC]
        xall[:, g, 9:18] = xprep[:, :, tbB:tbB + TC]
    return xall


def _unshard_out(main, tail):
    """main [BPC, NGRP, 128, FG], tail [BPC, NGRP, NT, FG] -> [BPC,O,N,T]"""
    out = np.empty((BPC, O, N, T), np.float32)
    for part, n0, n1 in ((main, 0, 128), (tail, 128, N)):
        r = part.reshape(BPC, NGRP, n1 - n0, NBLK, 2, 2, O)
        # dims: b, g, n, blk, ph, h, o ; tau = h*TH + g*TC + 2*blk + ph
        r = np.transpose(r, (0, 6, 2, 5, 1, 3, 4))  # b, o, n, h, g, blk, ph
        out[:, :, n0:n1, :] = r.reshape(BPC, O, n1 - n0, T)
    return out


def kernel(**inputs):
    from concourse.bass_utils import run_bass_kernel_spmd

    if "nc" not in _cache:
        _cache["nc"] = _build(1)
    nc = _cache["nc"]

    x = np.asarray(inputs["x"], np.float32)
    weights = _prep(inputs)
    xprep = _prep_x(x)
    in_maps = []
    for cid in range(NCORES):
        m = dict(weights)
        m["x"] = np.ascontiguousarray(xprep[cid * BPC:(cid + 1) * BPC])
        in_maps.append(m)
    res = run_bass_kernel_spmd(nc, in_maps, list(range(NCORES)), trace=False)
    out = np.concatenate(
        [_unshard_out(res.results[cid]["out"], res.results[cid]["out2"])
         for cid in range(NCORES)], axis=0)
    return out.astype(np.float32)
